# revision 2
# baseline (speedup 1.0000x reference)
"""Trainium2 Bass kernel for nn_LogicLayer (difflogic LogicLayer forward).

Computation (reference):
    w  = softmax(weights, axis=-1)            # [OUT, 16]
    c  = w @ GATE_M                           # [OUT, 4]
    a  = x[:, idx_a]; b = x[:, idx_b]         # [B, OUT] feature gathers
    out = c0 + c1*a + c2*b + c3*(a*b)

Strategy (8 NeuronCores, feature-parallel):
  - x uploaded transposed twice: xT8 (u8, q=rint(x*255)) for a-gathers,
    xT16 (bf16) for b-gathers. Each core: 2048 features x full batch.
  - Per 128-feature chunk: dma_gather a-row (4 KB u8) and b-row (8 KB
    bf16) per feature.
  - out is computed pre-scaled: o = SCALE*out + OFF in bf16, written to
    HBM as u8 via a casting SWDGE DMA (halves output HBM traffic).
    Host inverts: out = (q - OFF)/SCALE. SCALE/OFF chosen so o stays
    well inside [0,255] (out is a convex combination of gates in [0,1],
    so out in [0,1]).
  - Per chunk: u = uc0 + uc1*qa (ScalarE IDENT, free affine),
    v = vc2 + vc3*qa (DVE tensor_scalar, or ScalarE for balance),
    v *= b (DVE TT bf16 2x), o = v + u (DVE TT bf16 2x).
    All dequant/scale factors folded into the coefficients.
  - Gate coefficients computed on-device from `weights` (exp on ScalarE,
    strided-AP reductions on VectorE), as in the reference softmax path.
"""

import numpy as np

BATCH, IN_DIM, OUT_DIM = 4096, 16384, 16384
N_CORES = 8
F_CORE = OUT_DIM // N_CORES  # 2048 output features per core
P = 128

SCALE = 248.0  # out -> u8 code scale
OFF = 3.5  # keeps codes in [~1.5, ~253.5]: safe from wrap/saturate
CAST_GAMMA = 3.5  # host-side un-bias; 3.0 if the DMA cast truncates


def _build_nc(in_dim, feat_core, batch):
    """Build + compile the per-core Bass program (SPMD, identical cores)."""
    from contextlib import ExitStack

    import concourse.bacc as bacc
    import concourse.mybir as mybir
    import concourse.tile as tile

    F32 = mybir.dt.float32
    BF16 = mybir.dt.bfloat16
    U8 = mybir.dt.uint8
    I16 = mybir.dt.int16
    TT = feat_core // P  # feature chunks per core (16)
    mult = mybir.AluOpType.mult
    add = mybir.AluOpType.add
    subtract = mybir.AluOpType.subtract
    Ident = mybir.ActivationFunctionType.Identity

    nc = bacc.Bacc(
        "TRN2", target_bir_lowering=False, debug=False, num_swdge_queues=2
    )
    xT8 = nc.dram_tensor("xT8", [in_dim, batch], U8, kind="ExternalInput")
    xT16 = nc.dram_tensor("xT16", [in_dim, batch], BF16, kind="ExternalInput")
    w = nc.dram_tensor("w", [feat_core, 16], F32, kind="ExternalInput")
    # combined gather indices: per chunk, 128 idx_a then 128 idx_b
    idx = nc.dram_tensor("idx", [P, 2 * feat_core // 16], I16, kind="ExternalInput")
    outT8 = nc.dram_tensor("outT8", [feat_core, batch], U8, kind="ExternalOutput")

    with tile.TileContext(nc) as tc, ExitStack() as ctx:
        const_pool = ctx.enter_context(tc.tile_pool(name="const", bufs=1))
        g_pool = ctx.enter_context(tc.tile_pool(name="g", bufs=4))
        uv_pool = ctx.enter_context(tc.tile_pool(name="uv", bufs=4))

        # chunk-0 indices in their own tiny tile so the first gather only
        # waits on a 32 B/partition DMA, not the full index load
        idx0_sb = const_pool.tile([P, 16], I16, tag="idx0")
        nc.sync.dma_start(idx0_sb[:], idx[:, 0:16])
        idx_sb = const_pool.tile([P, 2 * feat_core // 16], I16, tag="idx")
        nc.sync.dma_start(idx_sb[:, 16:], idx[:, 16:])

        uc0 = const_pool.tile([P, TT], F32, tag="uc0")
        uc1 = const_pool.tile([P, TT], F32, tag="uc1")
        vc2 = const_pool.tile([P, TT], F32, tag="vc2")
        vc3 = const_pool.tile([P, TT], F32, tag="vc3")

        # ---------- gate coefficients ----------
        # Setup pool stays open for the kernel's lifetime (~5 KB/partition):
        # closing it would put a scope-exit barrier in front of the first
        # gather (~8 us of serialized lead-in).
        sp = ctx.enter_context(tc.tile_pool(name="setup", bufs=1))
        if True:
            w_sb = sp.tile([P, TT, 16], F32, tag="wsb")
            nc.sync.dma_start(w_sb[:], w[:].rearrange("(t p) g -> p t g", p=P))
            E = sp.tile([P, TT, 16], F32, tag="E")
            nc.scalar.activation(E[:], w_sb[:], mybir.ActivationFunctionType.Exp)

            su = sp.tile([P, TT], F32, tag="su")
            nc.vector.reduce_sum(su[:], E[:], axis=mybir.AxisListType.X)
            r = sp.tile([P, TT], F32, tag="r")
            nc.vector.reciprocal(r[:], su[:])

            c0u = sp.tile([P, TT], F32, tag="c0u")
            nc.vector.reduce_sum(c0u[:], E[:, :, 8:16], axis=mybir.AxisListType.X)

            E4 = E[:].rearrange("p t (g2 g1) -> p t g2 g1", g1=4)
            a1 = sp.tile([P, TT], F32, tag="a1")
            nc.vector.reduce_sum(a1[:], E4[:, :, 0:2, 2:4], axis=mybir.AxisListType.XY)
            b1 = sp.tile([P, TT], F32, tag="b1")
            nc.vector.reduce_sum(b1[:], E4[:, :, 2:4, 0:2], axis=mybir.AxisListType.XY)
            c1u = sp.tile([P, TT], F32, tag="c1u")
            nc.vector.tensor_tensor(c1u[:], a1[:], b1[:], op=subtract)

            a2 = sp.tile([P, TT], F32, tag="a2")
            nc.vector.reduce_sum(a2[:], E[:, :, 4:8], axis=mybir.AxisListType.X)
            b2 = sp.tile([P, TT], F32, tag="b2")
            nc.vector.reduce_sum(b2[:], E[:, :, 8:12], axis=mybir.AxisListType.X)
            c2u = sp.tile([P, TT], F32, tag="c2u")
            nc.vector.tensor_tensor(c2u[:], a2[:], b2[:], op=subtract)

            # c3 = (E1+E8) + (E11+E13) - (E2+E4) - (E7+E14) - 2*(E6-E9)
            def eg(g):
                return E[:, :, g : g + 1]

            p1 = sp.tile([P, TT, 1], F32, tag="p1")
            nc.vector.tensor_tensor(p1[:], eg(1), eg(8), op=add)
            p2 = sp.tile([P, TT, 1], F32, tag="p2")
            nc.vector.tensor_tensor(p2[:], eg(11), eg(13), op=add)
            n1 = sp.tile([P, TT, 1], F32, tag="n1")
            nc.vector.tensor_tensor(n1[:], eg(2), eg(4), op=add)
            n2 = sp.tile([P, TT, 1], F32, tag="n2")
            nc.vector.tensor_tensor(n2[:], eg(7), eg(14), op=add)
            d6 = sp.tile([P, TT, 1], F32, tag="d6")
            nc.vector.tensor_tensor(d6[:], eg(6), eg(9), op=subtract)
            pp = sp.tile([P, TT, 1], F32, tag="pp")
            nc.vector.tensor_tensor(pp[:], p1[:], p2[:], op=add)
            nn_ = sp.tile([P, TT, 1], F32, tag="nn")
            nc.vector.tensor_tensor(nn_[:], n1[:], n2[:], op=add)
            c3a = sp.tile([P, TT, 1], F32, tag="c3a")
            nc.vector.tensor_tensor(c3a[:], pp[:], nn_[:], op=subtract)
            c3u = sp.tile([P, TT, 1], F32, tag="c3u")
            nc.vector.scalar_tensor_tensor(
                c3u[:], d6[:], -2.0, c3a[:], op0=mult, op1=add
            )

            # fold normalization (r = 1/sum), the u8 dequant (a = qa/255)
            # and the output code scale (o = SCALE*out + OFF) into the
            # coefficients:
            #   u = uc0 + uc1*qa ; v = vc2 + vc3*qa ; o = u + v*b
            rS = sp.tile([P, TT], F32, tag="rS")
            nc.vector.tensor_scalar_mul(rS[:], r[:], SCALE)
            rS255 = sp.tile([P, TT], F32, tag="rS255")
            nc.vector.tensor_scalar_mul(rS255[:], r[:], SCALE / 255.0)
            uc0a = sp.tile([P, TT], F32, tag="uc0a")
            nc.vector.tensor_tensor(uc0a[:], c0u[:], rS[:], op=mult)
            nc.vector.tensor_scalar_add(uc0[:], uc0a[:], OFF)
            nc.vector.tensor_tensor(uc1[:], c1u[:], rS255[:], op=mult)
            nc.vector.tensor_tensor(vc2[:], c2u[:], rS[:], op=mult)
            nc.vector.tensor_tensor(vc3[:], c3u[:, :, 0], rS255[:], op=mult)

        # ---------- main gather + FMA loop ----------
        o_pool = ctx.enter_context(tc.tile_pool(name="o", bufs=4))
        V_ON_ACT = {1, 3, 5, 7, 9, 11, 13, 15}  # v-affine on ScalarE (balance)
        for ci in range(TT):
            # idx columns: first 8 are the 128 idx_a, next 8 the 128 idx_b
            isrc = idx0_sb if ci == 0 else idx_sb
            a_t = g_pool.tile([P, 1, batch], U8, tag="ga")
            nc.gpsimd.dma_gather(
                a_t[:], xT8[:], isrc[:, ci * 16 : ci * 16 + 8], 128, 128, batch,
                queue_num=ci % 2,
            )
            b_t = g_pool.tile([P, 1, batch], BF16, tag="gb16")
            nc.gpsimd.dma_gather(
                b_t[:], xT16[:], isrc[:, ci * 16 + 8 : ci * 16 + 16], 128, 128,
                batch, queue_num=(ci + 1) % 2,
            )
            b_v = b_t[:, 0, :]
            a_v = a_t[:, 0, :]
            cs = slice(ci, ci + 1)
            # u = uc0 + uc1*qa ; v = vc2 + vc3*qa
            u = uv_pool.tile([P, batch], BF16, tag="u")
            nc.scalar.activation(u[:], a_v, Ident, bias=uc0[:, cs], scale=uc1[:, cs])
            v = uv_pool.tile([P, batch], BF16, tag="v")
            if ci in V_ON_ACT:
                nc.scalar.activation(
                    v[:], a_v, Ident, bias=vc2[:, cs], scale=vc3[:, cs]
                )
            else:
                nc.vector.tensor_scalar(v[:], a_v, vc3[:, cs], vc2[:, cs], mult, add)
            # v = v*b, then o = v+u  (DVE, all-bf16)
            nc.vector.tensor_tensor(v[:], v[:], b_v, op=mult)
            o_t = o_pool.tile([P, batch], BF16, tag="o")
            nc.vector.tensor_tensor(o_t[:], v[:], u[:], op=add)
            # casting store: bf16 -> u8 (SWDGE)
            nc.gpsimd.dma_start(outT8[ci * P : (ci + 1) * P, :], o_t[:])

    nc.compile()
    return nc


def _pack_idx(idx_a, idx_b, feat_lo, feat_hi):
    """Host-side int16 gather-index buffer for one core.

    Per 128-feature chunk: 128 idx_a then 128 idx_b. dma_gather consumes
    index i from partition i%16, column i//16 (replicated across the 8
    groups of 16 partitions).
    """
    cols = []
    for f0 in range(feat_lo, feat_hi, P):
        ids = np.concatenate(
            [idx_a[f0 : f0 + P], idx_b[f0 : f0 + P]]
        ).astype(np.int16)
        blk = ids.reshape(16, 16)  # [col, partition-within-16]
        cols.append(np.tile(blk.T, (P // 16, 1)))  # [128, 16]
    return np.ascontiguousarray(np.concatenate(cols, axis=1))


_NC_CACHE = {}


def _get_nc():
    key = (IN_DIM, F_CORE, BATCH)
    if key not in _NC_CACHE:
        _NC_CACHE[key] = _build_nc(IN_DIM, F_CORE, BATCH)
    return _NC_CACHE[key]


TRACE = False  # set by dev harness to capture an NTFF profile
LAST_RESULT = None


def kernel(x, weights, idx_a, idx_b):
    global LAST_RESULT
    import ml_dtypes
    from concourse.bass_utils import run_bass_kernel_spmd

    x = np.asarray(x, dtype=np.float32)
    weights = np.asarray(weights, dtype=np.float32)
    idx_a = np.asarray(idx_a)
    idx_b = np.asarray(idx_b)

    nc = _get_nc()
    xT8 = np.ascontiguousarray(np.rint(x * 255.0).astype(np.uint8).T)
    xT16 = np.ascontiguousarray(x.astype(ml_dtypes.bfloat16).T)
    in_maps = []
    for k in range(N_CORES):
        lo, hi = k * F_CORE, (k + 1) * F_CORE
        in_maps.append(
            {
                "xT8": xT8,
                "xT16": xT16,
                "w": np.ascontiguousarray(weights[lo:hi]),
                "idx": _pack_idx(idx_a, idx_b, lo, hi),
            }
        )

    res = run_bass_kernel_spmd(nc, in_maps, list(range(N_CORES)), trace=TRACE)
    LAST_RESULT = res
    out = np.empty((BATCH, OUT_DIM), dtype=np.float32)
    for k in range(N_CORES):
        q = res.results[k]["outT8"].astype(np.float32)
        out[:, k * F_CORE : (k + 1) * F_CORE] = ((q - CAST_GAMMA) / SCALE).T
    return out


# revision 3
# speedup vs baseline: 1.4007x; 1.4007x over previous
"""Trainium2 Bass kernel for nn_LogicLayer (difflogic LogicLayer forward).

Computation (reference):
    w  = softmax(weights, axis=-1)            # [OUT, 16]
    c  = w @ GATE_M                           # [OUT, 4]
    a  = x[:, idx_a]; b = x[:, idx_b]         # [B, OUT] feature gathers
    out = c0 + c1*a + c2*b + c3*(a*b)

Strategy (8 NeuronCores, feature-parallel, division-form math):
  - x uploaded transposed twice: xT8 (u8, q=rint(x*255)) for a-gathers,
    xT16 (bf16) for b-gathers. Each core: 2048 features x full batch,
    16 chunks of 128 features.
  - Division form:  out = (c3*a + c2)*(b + c1/c3) + (c0 - c1*c2/c3).
    With the output code o = SCALE*out + OFF this becomes
        v' = S3*qa + S2          (ScalarE IDENT: free per-partition affine)
        b' = b + alpha           (DVE tensor_scalar add, bf16 4x mode —
                                  or ScalarE IDENT on some chunks, balance)
        m  = v' * b'             (DVE tensor_tensor, bf16 2x)
        o8 = m + beta' -> u8     (DVE tensor_scalar add, u8 out, 2x_2p)
    All three non-TT ops are per-partition-scalar affines, so the only
    expensive TT is the multiply, and the final pass emits u8 directly
    (plain HWDGE store, half the output bytes, no cast DMA).
  - Features where alpha = c1/c3 is ill-conditioned (|c3| small) are
    host-permuted into ONE "unsafe" chunk per core (chunk 15) computed
    with the classic form u + v*b (final TT at 1x, only 1 chunk pays).
    Host inverse-permutes the output columns.
  - Gate coefficients computed on-device from `weights` (exp on ScalarE,
    strided-AP reductions on VectorE), as in the reference softmax path.
"""

import numpy as np

BATCH, IN_DIM, OUT_DIM = 4096, 16384, 16384
N_CORES = 8
F_CORE = OUT_DIM // N_CORES  # 2048 output features per core
P = 128
N_CHUNKS = F_CORE // P  # 16
UNSAFE_CI = N_CHUNKS - 1  # chunk holding the ill-conditioned features

SCALE = 248.0  # out -> u8 code scale
OFF = 3.5  # keeps codes in [~1.5, ~253.5]: safe from wrap/saturate
CAST_GAMMA = 3.5  # host-side un-bias (3.0 if conversion truncates)

_GATE_M = np.array(
    [
        [0.0, 0.0, 0.0, 0.0], [0.0, 0.0, 0.0, 1.0],
        [0.0, 1.0, 0.0, -1.0], [0.0, 1.0, 0.0, 0.0],
        [0.0, 0.0, 1.0, -1.0], [0.0, 0.0, 1.0, 0.0],
        [0.0, 1.0, 1.0, -2.0], [0.0, 1.0, 1.0, -1.0],
        [1.0, -1.0, -1.0, 1.0], [1.0, -1.0, -1.0, 2.0],
        [1.0, 0.0, -1.0, 0.0], [1.0, 0.0, -1.0, 1.0],
        [1.0, -1.0, 0.0, 0.0], [1.0, -1.0, 0.0, 1.0],
        [1.0, 0.0, 0.0, -1.0], [1.0, 0.0, 0.0, 0.0],
    ],
    dtype=np.float64,
)


def _build_nc(in_dim, feat_core, batch):
    """Build + compile the per-core Bass program (SPMD, identical cores)."""
    from contextlib import ExitStack

    import concourse.bacc as bacc
    import concourse.mybir as mybir
    import concourse.tile as tile

    F32 = mybir.dt.float32
    BF16 = mybir.dt.bfloat16
    U8 = mybir.dt.uint8
    I16 = mybir.dt.int16
    TT = feat_core // P  # feature chunks per core (16)
    mult = mybir.AluOpType.mult
    add = mybir.AluOpType.add
    subtract = mybir.AluOpType.subtract
    Ident = mybir.ActivationFunctionType.Identity

    nc = bacc.Bacc(
        "TRN2", target_bir_lowering=False, debug=False, num_swdge_queues=2
    )
    xT8 = nc.dram_tensor("xT8", [in_dim, batch], U8, kind="ExternalInput")
    xT16 = nc.dram_tensor("xT16", [in_dim, batch], BF16, kind="ExternalInput")
    w = nc.dram_tensor("w", [feat_core, 16], F32, kind="ExternalInput")
    # combined gather indices: per chunk, 128 idx_a then 128 idx_b
    idx = nc.dram_tensor("idx", [P, 2 * feat_core // 16], I16, kind="ExternalInput")
    outT8 = nc.dram_tensor("outT8", [feat_core, batch], U8, kind="ExternalOutput")

    with tile.TileContext(nc) as tc, ExitStack() as ctx:
        const_pool = ctx.enter_context(tc.tile_pool(name="const", bufs=1))
        g_pool = ctx.enter_context(tc.tile_pool(name="g", bufs=4))
        uv_pool = ctx.enter_context(tc.tile_pool(name="uv", bufs=4))

        # chunk-0 indices in their own tiny tile so the first gather only
        # waits on a 32 B/partition DMA, not the full index load
        idx0_sb = const_pool.tile([P, 16], I16, tag="idx0")
        nc.sync.dma_start(idx0_sb[:], idx[:, 0:16])
        idx_sb = const_pool.tile([P, 2 * feat_core // 16], I16, tag="idx")
        nc.sync.dma_start(idx_sb[:, 16:], idx[:, 16:])

        sc2 = const_pool.tile([P, TT], F32, tag="sc2")  # SCALE*c2
        sc3 = const_pool.tile([P, TT], F32, tag="sc3")  # SCALE*c3/255
        alp = const_pool.tile([P, TT], F32, tag="alp")  # c1/c3
        bet = const_pool.tile([P, TT], F32, tag="bet")  # SCALE*(c0-c1c2/c3)+OFF
        uc0 = const_pool.tile([P, TT], F32, tag="uc0")  # SCALE*c0 + OFF
        uc1 = const_pool.tile([P, TT], F32, tag="uc1")  # SCALE*c1/255

        # ---------- gate coefficients ----------
        # Setup pool stays open for the kernel's lifetime (~5 KB/partition):
        # closing it would put a scope-exit barrier in front of the first
        # gather (~8 us of serialized lead-in).
        sp = ctx.enter_context(tc.tile_pool(name="setup", bufs=1))
        if True:
            w_sb = sp.tile([P, TT, 16], F32, tag="wsb")
            nc.sync.dma_start(w_sb[:], w[:].rearrange("(t p) g -> p t g", p=P))
            E = sp.tile([P, TT, 16], F32, tag="E")
            nc.scalar.activation(E[:], w_sb[:], mybir.ActivationFunctionType.Exp)

            su = sp.tile([P, TT], F32, tag="su")
            nc.vector.reduce_sum(su[:], E[:], axis=mybir.AxisListType.X)
            r = sp.tile([P, TT], F32, tag="r")
            nc.vector.reciprocal(r[:], su[:])

            c0u = sp.tile([P, TT], F32, tag="c0u")
            nc.vector.reduce_sum(c0u[:], E[:, :, 8:16], axis=mybir.AxisListType.X)

            E4 = E[:].rearrange("p t (g2 g1) -> p t g2 g1", g1=4)
            a1 = sp.tile([P, TT], F32, tag="a1")
            nc.vector.reduce_sum(a1[:], E4[:, :, 0:2, 2:4], axis=mybir.AxisListType.XY)
            b1 = sp.tile([P, TT], F32, tag="b1")
            nc.vector.reduce_sum(b1[:], E4[:, :, 2:4, 0:2], axis=mybir.AxisListType.XY)
            c1u = sp.tile([P, TT], F32, tag="c1u")
            nc.vector.tensor_tensor(c1u[:], a1[:], b1[:], op=subtract)

            a2 = sp.tile([P, TT], F32, tag="a2")
            nc.vector.reduce_sum(a2[:], E[:, :, 4:8], axis=mybir.AxisListType.X)
            b2 = sp.tile([P, TT], F32, tag="b2")
            nc.vector.reduce_sum(b2[:], E[:, :, 8:12], axis=mybir.AxisListType.X)
            c2u = sp.tile([P, TT], F32, tag="c2u")
            nc.vector.tensor_tensor(c2u[:], a2[:], b2[:], op=subtract)

            # c3 = (E1+E8) + (E11+E13) - (E2+E4) - (E7+E14) - 2*(E6-E9)
            def eg(g):
                return E[:, :, g : g + 1]

            p1 = sp.tile([P, TT, 1], F32, tag="p1")
            nc.vector.tensor_tensor(p1[:], eg(1), eg(8), op=add)
            p2 = sp.tile([P, TT, 1], F32, tag="p2")
            nc.vector.tensor_tensor(p2[:], eg(11), eg(13), op=add)
            n1 = sp.tile([P, TT, 1], F32, tag="n1")
            nc.vector.tensor_tensor(n1[:], eg(2), eg(4), op=add)
            n2 = sp.tile([P, TT, 1], F32, tag="n2")
            nc.vector.tensor_tensor(n2[:], eg(7), eg(14), op=add)
            d6 = sp.tile([P, TT, 1], F32, tag="d6")
            nc.vector.tensor_tensor(d6[:], eg(6), eg(9), op=subtract)
            pp = sp.tile([P, TT, 1], F32, tag="pp")
            nc.vector.tensor_tensor(pp[:], p1[:], p2[:], op=add)
            nn_ = sp.tile([P, TT, 1], F32, tag="nn")
            nc.vector.tensor_tensor(nn_[:], n1[:], n2[:], op=add)
            c3a = sp.tile([P, TT, 1], F32, tag="c3a")
            nc.vector.tensor_tensor(c3a[:], pp[:], nn_[:], op=subtract)
            c3u = sp.tile([P, TT, 1], F32, tag="c3u")
            nc.vector.scalar_tensor_tensor(
                c3u[:], d6[:], -2.0, c3a[:], op0=mult, op1=add
            )
            c3f = c3u[:, :, 0]

            # Folded coefficients. r = 1/sum(E) (softmax norm), qa = 255*a.
            #   v' = sc3*qa + sc2 = SCALE*(c3*a + c2)
            #   o  = v'*(b + alp) + bet ; alp = c1/c3 (r cancels)
            #   bet = SCALE*r*(c0u - alp*c2u) + OFF
            rS = sp.tile([P, TT], F32, tag="rS")
            nc.vector.tensor_scalar_mul(rS[:], r[:], SCALE)
            rS255 = sp.tile([P, TT], F32, tag="rS255")
            nc.vector.tensor_scalar_mul(rS255[:], r[:], SCALE / 255.0)
            nc.vector.tensor_tensor(sc2[:], c2u[:], rS[:], op=mult)
            nc.vector.tensor_tensor(sc3[:], c3f, rS255[:], op=mult)

            rc3 = sp.tile([P, TT], F32, tag="rc3")
            nc.vector.reciprocal(rc3[:], c3f)
            nc.vector.tensor_tensor(alp[:], c1u[:], rc3[:], op=mult)
            t1 = sp.tile([P, TT], F32, tag="t1")
            nc.vector.tensor_tensor(t1[:], alp[:], c2u[:], op=mult)
            t2 = sp.tile([P, TT], F32, tag="t2")
            nc.vector.tensor_tensor(t2[:], c0u[:], t1[:], op=subtract)
            t3 = sp.tile([P, TT], F32, tag="t3")
            nc.vector.tensor_tensor(t3[:], t2[:], rS[:], op=mult)
            nc.vector.tensor_scalar_add(bet[:], t3[:], OFF)

            # classic-form coefficients for the unsafe chunk
            uc0a = sp.tile([P, TT], F32, tag="uc0a")
            nc.vector.tensor_tensor(uc0a[:], c0u[:], rS[:], op=mult)
            nc.vector.tensor_scalar_add(uc0[:], uc0a[:], OFF)
            nc.vector.tensor_tensor(uc1[:], c1u[:], rS255[:], op=mult)

        # ---------- main gather + FMA loop ----------
        o_pool = ctx.enter_context(tc.tile_pool(name="o", bufs=4))
        B_ON_ACT = {1, 3, 5, 7, 9, 11, 13}  # b-shift on ScalarE (balance)
        for ci in range(TT):
            # idx columns: first 8 are the 128 idx_a, next 8 the 128 idx_b
            isrc = idx0_sb if ci == 0 else idx_sb
            a_t = g_pool.tile([P, 1, batch], U8, tag="ga")
            nc.gpsimd.dma_gather(
                a_t[:], xT8[:], isrc[:, ci * 16 : ci * 16 + 8], 128, 128, batch,
                queue_num=ci % 2,
            )
            b_t = g_pool.tile([P, 1, batch], BF16, tag="gb16")
            nc.gpsimd.dma_gather(
                b_t[:], xT16[:], isrc[:, ci * 16 + 8 : ci * 16 + 16], 128, 128,
                batch, queue_num=(ci + 1) % 2,
            )
            b_v = b_t[:, 0, :]
            a_v = a_t[:, 0, :]
            cs = slice(ci, ci + 1)
            # v' = sc3*qa + sc2  (ScalarE, free per-partition affine)
            v = uv_pool.tile([P, batch], BF16, tag="v")
            nc.scalar.activation(v[:], a_v, Ident, bias=sc2[:, cs], scale=sc3[:, cs])
            o8 = o_pool.tile([P, batch], U8, tag="o8")
            if ci != UNSAFE_CI:
                # b' = b + alpha
                bp = uv_pool.tile([P, batch], BF16, tag="bp")
                if ci in B_ON_ACT:
                    nc.scalar.activation(bp[:], b_v, Ident, bias=alp[:, cs])
                else:
                    nc.vector.tensor_scalar(bp[:], b_v, alp[:, cs], None, add)
                # m = v'*b' ; o8 = m + beta' (u8 out)
                nc.vector.tensor_tensor(v[:], v[:], bp[:], op=mult)
                nc.vector.tensor_scalar(o8[:], v[:], bet[:, cs], None, add)
            else:
                # ill-conditioned features: classic o = v'*b + u
                u = uv_pool.tile([P, batch], BF16, tag="bp")
                nc.scalar.activation(
                    u[:], a_v, Ident, bias=uc0[:, cs], scale=uc1[:, cs]
                )
                nc.vector.tensor_tensor(v[:], v[:], b_v, op=mult)
                nc.vector.tensor_tensor(o8[:], v[:], u[:], op=add)
            nc.sync.dma_start(outT8[ci * P : (ci + 1) * P, :], o8[:])

    nc.compile()
    return nc


def _pack_idx(idx_a, idx_b):
    """Host-side int16 gather-index buffer for one core.

    Per 128-feature chunk: 128 idx_a then 128 idx_b. dma_gather consumes
    index i from partition i%16, column i//16 (replicated across the 8
    groups of 16 partitions).
    """
    cols = []
    for f0 in range(0, len(idx_a), P):
        ids = np.concatenate(
            [idx_a[f0 : f0 + P], idx_b[f0 : f0 + P]]
        ).astype(np.int16)
        blk = ids.reshape(16, 16)  # [col, partition-within-16]
        cols.append(np.tile(blk.T, (P // 16, 1)))  # [128, 16]
    return np.ascontiguousarray(np.concatenate(cols, axis=1))


def _core_perm(weights, lo, hi):
    """Order this core's features: well-conditioned first, the 128 worst
    (by the bf16-magnitude metric of the division form) into the last
    chunk. Returns global feature indices in on-device order."""
    w = weights[lo:hi].astype(np.float64)
    e = np.exp(w - w.max(axis=1, keepdims=True))
    sm = e / e.sum(axis=1, keepdims=True)
    c = sm @ _GATE_M
    c0, c1, c2, c3 = c.T
    with np.errstate(divide="ignore", invalid="ignore"):
        al = c1 / c3
        vs = np.stack([SCALE * c2, SCALE * (c3 + c2)])  # v' at a in {0,1}
        bs = np.stack([al, 1.0 + al])  # b' at b in {0,1}
        m_max = np.max(np.abs(vs[:, None, :] * bs[None, :, :]), axis=(0, 1))
    metric = np.where(np.isfinite(m_max), m_max, np.inf)
    order = np.argsort(metric, kind="stable")
    # stable layout: safe features keep relative order; worst 128 go last
    return lo + np.concatenate([np.sort(order[: hi - lo - P]), np.sort(order[hi - lo - P :])])


_NC_CACHE = {}


def _get_nc():
    key = (IN_DIM, F_CORE, BATCH)
    if key not in _NC_CACHE:
        _NC_CACHE[key] = _build_nc(IN_DIM, F_CORE, BATCH)
    return _NC_CACHE[key]


TRACE = False  # set by dev harness to capture an NTFF profile
LAST_RESULT = None


def kernel(x, weights, idx_a, idx_b):
    global LAST_RESULT
    import ml_dtypes
    from concourse.bass_utils import run_bass_kernel_spmd

    x = np.asarray(x, dtype=np.float32)
    weights = np.asarray(weights, dtype=np.float32)
    idx_a = np.asarray(idx_a)
    idx_b = np.asarray(idx_b)

    nc = _get_nc()
    xT8 = np.ascontiguousarray(np.rint(x * 255.0).astype(np.uint8).T)
    xT16 = np.ascontiguousarray(x.astype(ml_dtypes.bfloat16).T)
    in_maps = []
    perms = []
    for k in range(N_CORES):
        lo, hi = k * F_CORE, (k + 1) * F_CORE
        perm = _core_perm(weights, lo, hi)
        perms.append(perm)
        in_maps.append(
            {
                "xT8": xT8,
                "xT16": xT16,
                "w": np.ascontiguousarray(weights[perm]),
                "idx": _pack_idx(idx_a[perm], idx_b[perm]),
            }
        )

    res = run_bass_kernel_spmd(nc, in_maps, list(range(N_CORES)), trace=TRACE)
    LAST_RESULT = res
    out = np.empty((BATCH, OUT_DIM), dtype=np.float32)
    for k in range(N_CORES):
        q = res.results[k]["outT8"].astype(np.float32)
        out[:, perms[k]] = ((q - CAST_GAMMA) / SCALE).T
    return out
